# revision 12
# baseline (speedup 1.0000x reference)
"""BatchBlur: depthwise 15x15 conv with per-sample kernels, reflection pad 7.

x: (32, 3, 512, 512) f32, kernel: (32, 15, 15) f32 -> out (32, 3, 512, 512) f32.

Strategy: pure data parallel over batch, 4 samples (12 channel-images) per
core on 8 cores. Host: reflection-pad x to (., 526, 526), cast to fp16, and
build dual-band matrices A[s, k, j, m]: for k<46, A = kern[s, k-m, 2j]; for
k>=46, A = kern[s, k-46-m, 2j+1] (band condition 0 <= dy < 15).

Device: FOUR images stream concurrently through the PE via 4-way column
tiling (128x32 tile mode, tile_position=(0, 32t)). Each image's rhs tile
holds its strip rows TWICE - partitions 0:46 at column offset 0 and
partitions 46:92 at column offset 1 - so one accumulating matmul covers TWO
horizontal taps (dx=2j lower band, dx=2j+1 upper band) for a 32-row strip:
  out[m, n] += sum_k A[k, j, m] * rhs[k, n + 2j]
Eight j-streams replace the fifteen per-tap matmuls, and the four column
tiles stream their rhs on independent XBUSes, so a slot of 4 matmuls costs
~one matmul's streaming time (~N/2.4GHz). LDWEIGHTS go to the per-tile
background weight buffer and hide under the streams.

512 = 16 strips x 32 rows exactly; rows 480..525 are the exact end of the
padded image, so there is no special-case last strip. Double-strip loads
(one DMA per band brings strips r0 and r0+32 in two free-dim blocks from an
overlapping strided DRAM view) halve the DMA issue rate. Output is stored
as fp16 (half the HBM write traffic) and upcast to f32 on the host; fp32
PSUM accumulation keeps relative error ~5e-4.
"""
import os
import sys

for _p in ("/opt/trn_rl_repo", "/root/.axon_site/_ro/trn_rl_repo"):
    if _p not in sys.path and os.path.isdir(_p):
        sys.path.insert(0, _p)

import numpy as np

import concourse.bass as bass
import concourse.mybir as mybir
import concourse.tile as tile
from concourse import bacc
from concourse.bass_utils import run_bass_kernel_spmd

L = 15           # blur kernel size
P = L // 2       # reflection pad
B, C, H, W = 32, 3, 512, 512
N_CORES = 8
BS = B // N_CORES            # samples per core
NIMG = BS * C                # channel images per core
HP, WP = H + 2 * P, W + 2 * P  # 526
M_STRIP = 32                 # output rows per strip (dual-band: 2*(32+14)=92)
K_GRP = M_STRIP + L - 1      # 46 input rows per band group
KK = 2 * K_GRP               # 92 rhs partitions
N_DX = (L + 1) // 2          # 8 streams (two taps each; last is single)
N_DU = H // (2 * M_STRIP)    # 8 double-strips per image
N_GRP = NIMG // 4            # 3 groups of 4 concurrent images
XBUFS = 8                    # rhs pool buffers (2 du of 4 images in flight)
N_WARMUP = 64                # dummy matmuls to release the HAM clock gate

F16 = mybir.dt.float16
F32 = mybir.dt.float32

_program_cache = None


def _build_program():
    nc = bacc.Bacc("TRN2", target_bir_lowering=False, debug=False)
    xp_d = nc.dram_tensor("xp", [NIMG, HP, WP], F16, kind="ExternalInput").ap()
    a_d = nc.dram_tensor("a", [BS, KK, N_DX, M_STRIP], F16,
                         kind="ExternalInput").ap()
    out_d = nc.dram_tensor("out", [NIMG, H, W], F16, kind="ExternalOutput").ap()

    def load_strip2(t, img, r0, beng, geng):
        # double-strip load: one DMA per band brings rows for strips r0 and
        # r0+32 (free-dim blocks 0:WP and WP:2*WP). The DRAM source is an
        # overlapping strided view (row stride WP, strip stride 32*WP) -
        # plain byte streams, legal for reads.
        base = (img * HP + r0) * WP
        beng.dma_start(
            out=t[0:K_GRP, :].rearrange("p (q c) -> p q c", c=WP),
            in_=bass.AP(xp_d.tensor, base,
                        [[WP, K_GRP], [M_STRIP * WP, 2], [1, WP]]))
        geng.dma_start(
            out=t[K_GRP:KK, :].rearrange(
                "p (q c) -> p q c", c=WP)[:, :, 0:WP - 1],
            in_=bass.AP(xp_d.tensor, base + 1,
                        [[WP, K_GRP], [M_STRIP * WP, 2], [1, WP - 1]]))

    with tile.TileContext(nc) as tc:
        with (
            tc.tile_pool(name="aconst", bufs=1) as apool,
            tc.tile_pool(name="warm", bufs=1) as wpool,
            tc.tile_pool(name="xin", bufs=XBUFS) as xpool,
            tc.tile_pool(name="oout", bufs=4) as opool,
            tc.tile_pool(name="psum", bufs=6, space="PSUM") as psum,
            tc.tile_pool(name="psumw", bufs=1, space="PSUM") as psumw,
        ):
            # HAM warm-up: a burst of full-array matmuls on a zeroed scratch
            # tile releases the PE clock gate while the first input DMAs are
            # in flight.
            wsrc = wpool.tile([128, 64], F16)
            nc.gpsimd.memset(wsrc[:], 0.0)
            wacc = psumw.tile([64, 64], F32)
            for _ in range(N_WARMUP):
                nc.tensor.matmul(wacc[:], wsrc[:, :64], wsrc[:], start=True,
                                 stop=True)

            # The upper-band DMAs write columns 0..524 of each strip block
            # only; the last column of each block is read (x 0.0 weight) by
            # the j=7 stream, so it must be finite. Zero it once per slot
            # (all 92 partitions - engine partition base must be 32-aligned;
            # band1's DMA overwrites its half with real data).
            for slot in range(XBUFS):
                t = xpool.tile([KK, 2 * WP], F16, tag="xp2", name="xz2")
                nc.gpsimd.memset(
                    t[:, :].rearrange("p (q c) -> p q c", c=WP)[:, :,
                                                               WP - 1:WP],
                    0.0)

            # first double-strip's image rows: issued before the A load so
            # the DMA queues deliver the first matmuls' dependencies
            # earliest. band1 on sync/scalar, band2 on gpsimd.
            xp_first = []
            for ti in range(4):
                t = xpool.tile([KK, 2 * WP], F16, tag="xp2", name=f"xpf{ti}")
                beng = nc.sync if ti < 2 else nc.scalar
                load_strip2(t, ti, 0, beng, nc.gpsimd)
                xp_first.append(t)

            # per-sample dual-band matrices: separate tiles => separate
            # dependency tracking; later samples load lazily
            a_t = [
                apool.tile([KK, N_DX, M_STRIP], F16, tag=f"a{s}",
                           name=f"a{s}")
                for s in range(BS)
            ]
            nc.sync.dma_start(out=a_t[0][:], in_=a_d[0])
            nc.sync.dma_start(out=a_t[1][:], in_=a_d[1])

            a_loaded = 1
            # 32-col-tiled (q-mode) matmuls are invisible to the PE HAM
            # activity monitor: without help the clock gate re-throttles to
            # 1.2 GHz ~3.4us after the warm-up burst and the whole kernel
            # runs at half clock. Inject a tiny VISIBLE (128x64 tile mode)
            # matmul every KEEPALIVE q-slots (~2.6us warm) to keep the
            # activity window busy. Costs two tiling-mode drains each time.
            def keepalive(dep):
                # rhs reads `dep` (a just-written SBUF slice) so the
                # dependency-driven scheduler cannot hoist this to t=0.
                nc.tensor.matmul(wacc[:, 0:1], wsrc[:, :64], dep,
                                 start=True, stop=True)

            for grp in range(N_GRP):
                imgs = [4 * grp + t for t in range(4)]
                smps = [img // C for img in imgs]
                # prefetch the A matrices the NEXT group needs
                for s_need in set((4 * grp + 4 + t) // C for t in range(4)):
                    if s_need < BS and s_need > a_loaded:
                        nc.sync.dma_start(out=a_t[s_need][:], in_=a_d[s_need])
                        a_loaded = s_need

                for du in range(N_DU):
                    r0 = 2 * M_STRIP * du
                    if grp == 0 and du == 0:
                        xt = xp_first
                    else:
                        xt = []
                        for ti in range(4):
                            t = xpool.tile([KK, 2 * WP], F16, tag="xp2",
                                           name=f"x{ti}")
                            beng = nc.sync if ti < 2 else nc.scalar
                            load_strip2(t, imgs[ti], r0, beng, nc.gpsimd)
                            xt.append(t)
                    o_t = opool.tile([128, 2 * W], F16)
                    for sub in range(2):
                        cbase = sub * WP
                        acc = psum.tile([128, W], F32)
                        # all 8 streams use K=92 (j=7's upper band is zero
                        # weights) - a K=46 stream would switch the PE
                        # tiling mode and pay a drain twice per strip
                        for j in range(N_DX):
                            for ti in range(4):
                                nc.tensor.matmul(
                                    acc[32 * ti:32 * ti + M_STRIP],
                                    a_t[smps[ti]][:, j, :],
                                    xt[ti][:, cbase + 2 * j:cbase + 2 * j + W],
                                    start=(j == 0),
                                    stop=(j == N_DX - 1),
                                    tile_position=(0, 32 * ti),
                                )
                        nc.vector.tensor_copy(
                            out=o_t[:, sub * W:(sub + 1) * W],
                            in_=acc[:])
                        keepalive(o_t[:, sub * W:sub * W + 1])
                    # one store per image covers both strips (64 contiguous
                    # output rows; non-overlapping views)
                    for ti in range(4):
                        dv = out_d[imgs[ti], r0:r0 + 2 * M_STRIP, :].rearrange(
                            "(q p) c -> p q c", q=2)
                        sv = o_t[32 * ti:32 * ti + M_STRIP, :].rearrange(
                            "p (q c) -> p q c", c=W)
                        oeng = nc.sync if ti < 2 else nc.scalar
                        oeng.dma_start(out=dv, in_=sv)
    nc.compile()
    return nc


def prepare_in_maps(x: np.ndarray, kern: np.ndarray) -> list:
    # host-side reflection pad, cast to fp16 for half the DMA bytes
    xp = np.pad(x, ((0, 0), (0, 0), (P, P), (P, P)), mode="reflect")
    xp = np.ascontiguousarray(
        xp.reshape(B * C, HP, WP).astype(np.float16))

    # dual-band matrices: lower band = even taps, upper band = odd taps
    kern16 = kern.astype(np.float16)
    a_all = np.zeros((B, KK, N_DX, M_STRIP), dtype=np.float16)
    m_idx = np.arange(M_STRIP)
    for dy in range(L):
        a_all[:, m_idx + dy, :, m_idx] = kern16[:, dy, 0::2]
        a_all[:, K_GRP + m_idx + dy, :L // 2, m_idx] = kern16[:, dy, 1::2]

    return [
        {
            "xp": xp[c * NIMG:(c + 1) * NIMG],
            "a": a_all[c * BS:(c + 1) * BS],
        }
        for c in range(N_CORES)
    ]


def kernel(x: np.ndarray, kernel: np.ndarray) -> np.ndarray:
    global _program_cache
    x = np.asarray(x, dtype=np.float32)
    kern = np.asarray(kernel, dtype=np.float32)

    in_maps = prepare_in_maps(x, kern)
    if _program_cache is None:
        _program_cache = _build_program()
    nc = _program_cache

    res = run_bass_kernel_spmd(nc, in_maps, core_ids=list(range(N_CORES)))
    out = np.concatenate([r["out"] for r in res.results], axis=0)
    return out.reshape(B, C, H, W).astype(np.float32)


# revision 17
# speedup vs baseline: 1.6170x; 1.6170x over previous
"""BatchBlur: depthwise 15x15 conv with per-sample kernels, reflection pad 7.

x: (32, 3, 512, 512) f32, kernel: (32, 15, 15) f32 -> out (32, 3, 512, 512) f32.

Strategy: pure data parallel over batch, 4 samples (12 channel-images) per
core on 8 cores. Host: reflection-pad x to (., 526, 526), cast to fp16, and
build dual-band matrices A[s, k, j, m]: for k<64, A = kern[s, k-m, 2j]; for
k>=64, A = kern[s, k-64-m, 2j+1] (band condition 0 <= dy < 15, m < 32 so
rows 46:64 of each band are zero).

Device: FOUR images stream concurrently through the PE via 4-way column
tiling (128x32 tile mode, tile_position=(0, 32t)), each on its own XBUS.
Each image's rhs tile holds 64 strip rows TWICE - partitions 0:64 at column
offset 0 and partitions 64:128 at column offset 1 (two DMAs straight from
DRAM, 64-partition aligned for full DMA speed) - so a single accumulating
matmul covers TWO horizontal taps (dx=2j in the lower band, dx=2j+1 in the
upper band) for a 32-row strip:
  out[m, n] += sum_k A[k, j, m] * rhs[k, n + 2j]
Eight j-streams replace the fifteen per-tap matmuls. 512 = 16 strips x 32
rows exactly (no special-case last strip; the last strip's 64-row band ends
exactly at padded row 525). Double-strip loads (one DMA per band brings
strips r0 and r0+32 in two free-dim blocks from an overlapping strided DRAM
view) halve the DMA issue rate. Output is stored as fp16 (half the HBM
write traffic) and upcast to f32 on the host; fp32 PSUM accumulation keeps
relative error ~6e-4.
"""
import os
import sys

for _p in ("/opt/trn_rl_repo", "/root/.axon_site/_ro/trn_rl_repo"):
    if _p not in sys.path and os.path.isdir(_p):
        sys.path.insert(0, _p)

import numpy as np

import concourse.bass as bass
import concourse.mybir as mybir
import concourse.tile as tile
from concourse import bacc
from concourse.bass_utils import run_bass_kernel_spmd

L = 15           # blur kernel size
P = L // 2       # reflection pad
B, C, H, W = 32, 3, 512, 512
N_CORES = 8
BS = B // N_CORES            # samples per core
NIMG = BS * C                # channel images per core
HP, WP = H + 2 * P, W + 2 * P  # 526
HPP = 544        # DRAM rows per image: HP + 18 zero slack rows so the
                 # last double-strip's 64-row aligned bands stay in bounds
                 # (max row read = 448 + 63 + 32 = 543; contents are
                 # multiplied by zero A-weights)
M_STRIP = 32                 # output rows per strip
K_GRP = 64                   # input rows per band (aligned; 46 used)
KK = 2 * K_GRP               # 128 rhs partitions
N_DX = (L + 1) // 2          # 8 streams (two taps each; last is single)
N_DU = H // (2 * M_STRIP)    # 8 double-strips per image
N_GRP = NIMG // 4            # 3 groups of 4 concurrent images
XBUFS = 8                    # rhs pool buffers (2 du of 4 images in flight)
N_WARMUP = 64                # dummy matmuls to release the HAM clock gate

F16 = mybir.dt.float16
F32 = mybir.dt.float32

_program_cache = None


def _build_program():
    nc = bacc.Bacc("TRN2", target_bir_lowering=False, debug=False)
    xp_d = nc.dram_tensor("xp", [NIMG, HPP, WP], F16,
                          kind="ExternalInput").ap()
    a_d = nc.dram_tensor("a", [BS, KK, N_DX, M_STRIP], F16,
                         kind="ExternalInput").ap()
    out_d = nc.dram_tensor("out", [NIMG, H, W], F16, kind="ExternalOutput").ap()

    def load_strip2(t, img, r0, beng):
        # double-strip load: one DMA per band brings rows for strips r0 and
        # r0+32 (free-dim blocks 0:WP and WP:2*WP). The DRAM source is an
        # overlapping strided view (row stride WP, strip stride 32*WP) -
        # plain byte streams, legal for reads. band1 on beng, band2 (same
        # rows, column offset 1, upper 64 partitions) on gpsimd.
        base = (img * HPP + r0) * WP
        beng.dma_start(
            out=t[0:K_GRP, :].rearrange("p (q c) -> p q c", c=WP),
            in_=bass.AP(xp_d.tensor, base,
                        [[WP, K_GRP], [M_STRIP * WP, 2], [1, WP]]))
        nc.gpsimd.dma_start(
            out=t[K_GRP:KK, :].rearrange(
                "p (q c) -> p q c", c=WP)[:, :, 0:WP - 1],
            in_=bass.AP(xp_d.tensor, base + 1,
                        [[WP, K_GRP], [M_STRIP * WP, 2], [1, WP - 1]]))

    with tile.TileContext(nc) as tc:
        with (
            tc.tile_pool(name="aconst", bufs=1) as apool,
            tc.tile_pool(name="warm", bufs=1) as wpool,
            tc.tile_pool(name="xin", bufs=XBUFS) as xpool,
            tc.tile_pool(name="oout", bufs=4) as opool,
            tc.tile_pool(name="psum", bufs=6, space="PSUM") as psum,
            tc.tile_pool(name="psumw", bufs=1, space="PSUM") as psumw,
        ):
            # HAM warm-up: a burst of full-array matmuls on a zeroed scratch
            # tile releases the PE clock gate while the first input DMAs are
            # in flight.
            wsrc = wpool.tile([128, 64], F16)
            nc.gpsimd.memset(wsrc[:], 0.0)
            wacc = psumw.tile([64, 64], F32)
            for _ in range(N_WARMUP):
                nc.tensor.matmul(wacc[:], wsrc[:, :64], wsrc[:], start=True,
                                 stop=True)

            # The upper-band DMAs write columns 0..524 of each strip block
            # only; the last column of each block is read (x 0.0 weight) by
            # the j=7 stream, so it must be finite. Zero it once per slot.
            for slot in range(XBUFS):
                t = xpool.tile([KK, 2 * WP], F16, tag="xp2", name="xz2")
                nc.gpsimd.memset(
                    t[K_GRP:KK, :].rearrange(
                        "p (q c) -> p q c", c=WP)[:, :, WP - 1:WP],
                    0.0)

            # first double-strip's image rows: issued before the A load so
            # the DMA queues deliver the first matmuls' dependencies
            # earliest. band1 on sync/scalar, band2 on gpsimd.
            xp_first = []
            for ti in range(4):
                t = xpool.tile([KK, 2 * WP], F16, tag="xp2", name=f"xpf{ti}")
                load_strip2(t, ti, 0, nc.sync if ti < 2 else nc.scalar)
                xp_first.append(t)

            # per-sample dual-band matrices: separate tiles => separate
            # dependency tracking; later samples load lazily
            a_t = [
                apool.tile([KK, N_DX, M_STRIP], F16, tag=f"a{s}",
                           name=f"a{s}")
                for s in range(BS)
            ]
            nc.sync.dma_start(out=a_t[0][:], in_=a_d[0])
            nc.sync.dma_start(out=a_t[1][:], in_=a_d[1])

            a_loaded = 1
            for grp in range(N_GRP):
                imgs = [4 * grp + t for t in range(4)]
                smps = [img // C for img in imgs]
                # prefetch the A matrices the NEXT group needs
                for s_need in sorted(set(
                        (4 * grp + 4 + t) // C for t in range(4))):
                    if s_need < BS and s_need > a_loaded:
                        nc.sync.dma_start(out=a_t[s_need][:], in_=a_d[s_need])
                        a_loaded = s_need

                for du in range(N_DU):
                    r0 = 2 * M_STRIP * du
                    if grp == 0 and du == 0:
                        xt = xp_first
                    else:
                        xt = []
                        for ti in range(4):
                            t = xpool.tile([KK, 2 * WP], F16, tag="xp2",
                                           name=f"x{ti}")
                            load_strip2(t, imgs[ti], r0,
                                        nc.sync if ti < 2 else nc.scalar)
                            xt.append(t)
                    o_t = opool.tile([128, 2 * W], F16)
                    for sub in range(2):
                        cbase = sub * WP
                        acc = psum.tile([128, W], F32)
                        # all 8 streams use K=128 (j=7's upper band is zero
                        # weights) - a shorter-K stream would switch the PE
                        # tiling mode and pay a drain twice per strip
                        for j in range(N_DX):
                            for ti in range(4):
                                nc.tensor.matmul(
                                    acc[32 * ti:32 * ti + M_STRIP],
                                    a_t[smps[ti]][:, j, :],
                                    xt[ti][:, cbase + 2 * j:cbase + 2 * j + W],
                                    start=(j == 0),
                                    stop=(j == N_DX - 1),
                                    tile_position=(0, 32 * ti),
                                )
                        nc.vector.tensor_copy(
                            out=o_t[:, sub * W:(sub + 1) * W],
                            in_=acc[:])
                    # one store per image covers both strips (64 contiguous
                    # output rows; non-overlapping views)
                    for ti in range(4):
                        dv = out_d[imgs[ti], r0:r0 + 2 * M_STRIP, :].rearrange(
                            "(q p) c -> p q c", q=2)
                        sv = o_t[32 * ti:32 * ti + M_STRIP, :].rearrange(
                            "p (q c) -> p q c", c=W)
                        oeng = nc.sync if ti < 2 else nc.scalar
                        oeng.dma_start(out=dv, in_=sv)
    nc.compile()
    return nc


def prepare_in_maps(x: np.ndarray, kern: np.ndarray) -> list:
    # host-side reflection pad, cast to fp16 for half the DMA bytes;
    # HPP - HP zero slack rows keep the aligned 64-row band loads in bounds
    xp = np.pad(x, ((0, 0), (0, 0), (P, P), (P, P)), mode="reflect")
    xpp = np.zeros((B * C, HPP, WP), dtype=np.float16)
    xpp[:, :HP] = xp.reshape(B * C, HP, WP).astype(np.float16)
    xp = xpp

    # dual-band matrices: lower band = even taps, upper band = odd taps
    kern16 = kern.astype(np.float16)
    a_all = np.zeros((B, KK, N_DX, M_STRIP), dtype=np.float16)
    m_idx = np.arange(M_STRIP)
    for dy in range(L):
        a_all[:, m_idx + dy, :, m_idx] = kern16[:, dy, 0::2]
        a_all[:, K_GRP + m_idx + dy, :L // 2, m_idx] = kern16[:, dy, 1::2]

    return [
        {
            "xp": xp[c * NIMG:(c + 1) * NIMG],
            "a": a_all[c * BS:(c + 1) * BS],
        }
        for c in range(N_CORES)
    ]


def kernel(x: np.ndarray, kernel: np.ndarray) -> np.ndarray:
    global _program_cache
    x = np.asarray(x, dtype=np.float32)
    kern = np.asarray(kernel, dtype=np.float32)

    in_maps = prepare_in_maps(x, kern)
    if _program_cache is None:
        _program_cache = _build_program()
    nc = _program_cache

    res = run_bass_kernel_spmd(nc, in_maps, core_ids=list(range(N_CORES)))
    out = np.concatenate([r["out"] for r in res.results], axis=0)
    return out.reshape(B, C, H, W).astype(np.float32)


# revision 21
# speedup vs baseline: 2.3145x; 1.4313x over previous
"""BatchBlur: depthwise 15x15 conv with per-sample kernels, reflection pad 7.

x: (32, 3, 512, 512) f32, kernel: (32, 15, 15) f32 -> out (32, 3, 512, 512) f32.

Strategy: pure data parallel over batch, 4 samples (12 channel-images) per
core on 8 cores. Host: reflection-pad x to (., 526, 526), cast to fp16, and
build dual-band matrices A[s, k, j, m]: for k<64, A = kern[s, k-m, 2j]; for
k>=64, A = kern[s, k-64-m, 2j+1] (band condition 0 <= dy < 15, m < 32 so
rows 46:64 of each band are zero).

Device: FOUR images stream concurrently through the PE via 4-way column
tiling (128x32 tile mode, tile_position=(0, 32t)), each on its own XBUS.
Each image's rhs tile holds 64 strip rows TWICE - partitions 0:64 at column
offset 0 and partitions 64:128 at column offset 1 (two DMAs straight from
DRAM, 64-partition aligned for full DMA speed) - so a single accumulating
matmul covers TWO horizontal taps (dx=2j in the lower band, dx=2j+1 in the
upper band) for a 32-row strip:
  out[m, n] += sum_k A[k, j, m] * rhs[k, n + 2j]
Eight j-streams replace the fifteen per-tap matmuls. 512 = 16 strips x 32
rows exactly (no special-case last strip; the last strip's 64-row band ends
exactly at padded row 525). Double-strip loads (one DMA per band brings
strips r0 and r0+32 in two free-dim blocks from an overlapping strided DRAM
view) halve the DMA issue rate. Output is stored as fp16 (half the HBM
write traffic) and upcast to f32 on the host; fp32 PSUM accumulation keeps
relative error ~6e-4.
"""
import os
import sys

for _p in ("/opt/trn_rl_repo", "/root/.axon_site/_ro/trn_rl_repo"):
    if _p not in sys.path and os.path.isdir(_p):
        sys.path.insert(0, _p)

import numpy as np

import concourse.bass as bass
import concourse.mybir as mybir
import concourse.tile as tile
from concourse import bacc
from concourse.bass_utils import run_bass_kernel_spmd

L = 15           # blur kernel size
P = L // 2       # reflection pad
B, C, H, W = 32, 3, 512, 512
N_CORES = 8
BS = B // N_CORES            # samples per core
NIMG = BS * C                # channel images per core
HP, WP = H + 2 * P, W + 2 * P  # 526
HPP = 544        # DRAM rows per image: HP + 18 zero slack rows so the
                 # last double-strip's 48-row aligned bands stay in bounds
                 # (max row read = 448 + 47 + 32 = 527; contents are
                 # multiplied by zero A-weights)
M_STRIP = 32                 # output rows per strip
K_GRP = 48                   # input rows per band (8-aligned; 46 used)
KK = 2 * K_GRP               # 96 rhs partitions
N_DX = (L + 1) // 2          # 8 streams (two taps each; last is single)
N_DU = H // (2 * M_STRIP)    # 8 double-strips per image
N_GRP = NIMG // 4            # 3 groups of 4 concurrent images
XBUFS = 12                   # rhs pool buffers (3 du of 4 images in flight)
N_WARMUP = 64                # dummy matmuls to release the HAM clock gate

F16 = mybir.dt.float16
F32 = mybir.dt.float32

_program_cache = None


def _build_program():
    nc = bacc.Bacc("TRN2", target_bir_lowering=False, debug=False)
    xp_d = nc.dram_tensor("xp", [NIMG, HPP, WP], F16,
                          kind="ExternalInput").ap()
    a_d = nc.dram_tensor("a", [BS, KK, N_DX, M_STRIP], F16,
                         kind="ExternalInput").ap()
    out_d = nc.dram_tensor("out", [NIMG, H, W], F16, kind="ExternalOutput").ap()

    def load_strip2(t, img, r0, beng):
        # double-strip load: one DMA per band brings rows for strips r0 and
        # r0+32 (free-dim blocks 0:WP and WP:2*WP). The DRAM source is an
        # overlapping strided view (row stride WP, strip stride 32*WP) -
        # plain byte streams, legal for reads. band1 on beng, band2 (same
        # rows, column offset 1, upper 64 partitions) on gpsimd.
        base = (img * HPP + r0) * WP
        beng.dma_start(
            out=t[0:K_GRP, :].rearrange("p (q c) -> p q c", c=WP),
            in_=bass.AP(xp_d.tensor, base,
                        [[WP, K_GRP], [M_STRIP * WP, 2], [1, WP]]))
        nc.gpsimd.dma_start(
            out=t[K_GRP:KK, :].rearrange(
                "p (q c) -> p q c", c=WP)[:, :, 0:WP - 1],
            in_=bass.AP(xp_d.tensor, base + 1,
                        [[WP, K_GRP], [M_STRIP * WP, 2], [1, WP - 1]]))

    with tile.TileContext(nc) as tc:
        with (
            tc.tile_pool(name="aconst", bufs=1) as apool,
            tc.tile_pool(name="warm", bufs=1) as wpool,
            tc.tile_pool(name="xin", bufs=XBUFS) as xpool,
            tc.tile_pool(name="oout", bufs=4) as opool,
            tc.tile_pool(name="psum", bufs=6, space="PSUM") as psum,
            tc.tile_pool(name="psumw", bufs=1, space="PSUM") as psumw,
        ):
            # HAM warm-up: a burst of full-array matmuls on a zeroed scratch
            # tile releases the PE clock gate while the first input DMAs are
            # in flight.
            wsrc = wpool.tile([128, 64], F16)
            nc.gpsimd.memset(wsrc[:], 0.0)
            wacc = psumw.tile([64, 64], F32)
            for _ in range(N_WARMUP):
                nc.tensor.matmul(wacc[:], wsrc[:, :64], wsrc[:], start=True,
                                 stop=True)

            # The upper-band DMAs write columns 0..524 of each strip block
            # only; the last column of each block is read (x 0.0 weight) by
            # the j=7 stream, so it must be finite. Zero it once per slot
            # across all partitions (engine partition base must be
            # 32-aligned; band1's DMA rewrites its half with real data).
            for slot in range(XBUFS):
                t = xpool.tile([KK, 2 * WP], F16, tag="xp2", name="xz2")
                nc.gpsimd.memset(
                    t[:, :].rearrange(
                        "p (q c) -> p q c", c=WP)[:, :, WP - 1:WP],
                    0.0)

            # first double-strip's image rows: issued before the A load so
            # the DMA queues deliver the first matmuls' dependencies
            # earliest. band1 on sync/scalar, band2 on gpsimd.
            xp_first = []
            for ti in range(4):
                t = xpool.tile([KK, 2 * WP], F16, tag="xp2", name=f"xpf{ti}")
                load_strip2(t, ti, 0, nc.sync if ti < 2 else nc.scalar)
                xp_first.append(t)

            # per-sample dual-band matrices: separate tiles => separate
            # dependency tracking; later samples load lazily
            a_t = [
                apool.tile([KK, N_DX, M_STRIP], F16, tag=f"a{s}",
                           name=f"a{s}")
                for s in range(BS)
            ]
            nc.sync.dma_start(out=a_t[0][:], in_=a_d[0])
            nc.sync.dma_start(out=a_t[1][:], in_=a_d[1])

            a_loaded = 1
            for grp in range(N_GRP):
                imgs = [4 * grp + t for t in range(4)]
                smps = [img // C for img in imgs]
                # prefetch the A matrices the NEXT group needs
                for s_need in sorted(set(
                        (4 * grp + 4 + t) // C for t in range(4))):
                    if s_need < BS and s_need > a_loaded:
                        nc.sync.dma_start(out=a_t[s_need][:], in_=a_d[s_need])
                        a_loaded = s_need

                for du in range(N_DU):
                    r0 = 2 * M_STRIP * du
                    if grp == 0 and du == 0:
                        xt = xp_first
                    else:
                        xt = []
                        for ti in range(4):
                            t = xpool.tile([KK, 2 * WP], F16, tag="xp2",
                                           name=f"x{ti}")
                            load_strip2(t, imgs[ti], r0,
                                        nc.sync if ti < 2 else nc.scalar)
                            xt.append(t)
                    o_t = opool.tile([128, 2 * W], F16)
                    for sub in range(2):
                        cbase = sub * WP
                        acc = psum.tile([128, W], F32)
                        # all 8 streams use K=128 (j=7's upper band is zero
                        # weights) - a shorter-K stream would switch the PE
                        # tiling mode and pay a drain twice per strip
                        for j in range(N_DX):
                            for ti in range(4):
                                nc.tensor.matmul(
                                    acc[32 * ti:32 * ti + M_STRIP],
                                    a_t[smps[ti]][:, j, :],
                                    xt[ti][:, cbase + 2 * j:cbase + 2 * j + W],
                                    start=(j == 0),
                                    stop=(j == N_DX - 1),
                                    tile_position=(0, 32 * ti),
                                )
                        nc.vector.tensor_copy(
                            out=o_t[:, sub * W:(sub + 1) * W],
                            in_=acc[:])
                    # one store per image covers both strips (64 contiguous
                    # output rows; non-overlapping views)
                    for ti in range(4):
                        dv = out_d[imgs[ti], r0:r0 + 2 * M_STRIP, :].rearrange(
                            "(q p) c -> p q c", q=2)
                        sv = o_t[32 * ti:32 * ti + M_STRIP, :].rearrange(
                            "p (q c) -> p q c", c=W)
                        oeng = nc.sync if ti < 2 else nc.scalar
                        oeng.dma_start(out=dv, in_=sv)
    nc.compile()
    return nc


def prepare_in_maps(x: np.ndarray, kern: np.ndarray) -> list:
    # host-side reflection pad, cast to fp16 for half the DMA bytes;
    # HPP - HP zero slack rows keep the aligned 64-row band loads in bounds
    xp = np.pad(x, ((0, 0), (0, 0), (P, P), (P, P)), mode="reflect")
    xpp = np.zeros((B * C, HPP, WP), dtype=np.float16)
    xpp[:, :HP] = xp.reshape(B * C, HP, WP).astype(np.float16)
    xp = xpp

    # dual-band matrices: lower band = even taps, upper band = odd taps
    kern16 = kern.astype(np.float16)
    a_all = np.zeros((B, KK, N_DX, M_STRIP), dtype=np.float16)
    m_idx = np.arange(M_STRIP)
    for dy in range(L):
        a_all[:, m_idx + dy, :, m_idx] = kern16[:, dy, 0::2]
        a_all[:, K_GRP + m_idx + dy, :L // 2, m_idx] = kern16[:, dy, 1::2]

    return [
        {
            "xp": xp[c * NIMG:(c + 1) * NIMG],
            "a": a_all[c * BS:(c + 1) * BS],
        }
        for c in range(N_CORES)
    ]


def kernel(x: np.ndarray, kernel: np.ndarray) -> np.ndarray:
    global _program_cache
    x = np.asarray(x, dtype=np.float32)
    kern = np.asarray(kernel, dtype=np.float32)

    in_maps = prepare_in_maps(x, kern)
    if _program_cache is None:
        _program_cache = _build_program()
    nc = _program_cache

    res = run_bass_kernel_spmd(nc, in_maps, core_ids=list(range(N_CORES)))
    out = np.concatenate([r["out"] for r in res.results], axis=0)
    return out.reshape(B, C, H, W).astype(np.float32)
